# revision 18
# baseline (speedup 1.0000x reference)
"""Trainium2 Bass kernel for nn_Encoder_24266565222656.

Reference computation (per batch b):
  conv[t,f]  = relu(sum_{w,d} x[t+w,d] * K[w,d,f] + cb[f])        (T_c=256, F=256)
  q = conv @ W1 + b1 ; v = conv @ W2 + b2                          (U=128)
  score[t,j] = sum_u V[u] * tanh(q[t,u] + v[j,u])                  (+bV, cancels in softmax)
  attn = softmax_j(score)
  out[b',t',f] = conv[b',t',f] * attn[t'%16, b'*16 + t'//16, f]    (the reshape scramble)

Key idea: tanh(x) ~= c*x + sum_m a_m sin(m*om1*x) (harmonic sine fit,
minimax 3.5e-3 on |x|<=7 at om1=0.65, M=5).  Each sine factorizes exactly:
  sin(w(q+v)) = sin(w q)cos(w v) + cos(w q)sin(w v)
so score becomes 2M+1 dense (128t x 128u x 256j) PE matmuls per batch instead
of 134M scalar tanh evals.  The linear term's q-part is constant over j
(softmax-invariant, dropped); its v-part is one ones-lhsT matmul.

Feature generation: the ACT Sin spline is valid on ~[-3.9, 3.9].
  m=1 sin: |om1*y| <= 3.6 -> direct ACT Sin on q/v (no range reduction).
  m=1 cos and m=5 sin/cos: range-reduced w = y - P*round(y/P + 1/8) via a
  DVE int32 round + cody_waite_cascade, then ACT Sin(scale, bias).
  m=2,3,4: DVE multiple-angle products (exact identities, bf16):
    sin2 = 2*s1*c1        cos2 = 1-2*s1^2
    sin3 = s1*(3-4*s1^2)  cos3 = c1*(1-4*s1^2)
    sin4 = 4*(s1*c1)*cos2 cos4 = 1-8*(s1*c1)^2
  The 2x/4x scalings are folded into the per-m aV fold constants (kappa).
  s1^2 and (s1*c1)^2 are computed with ACT Square; the rest on DVE.

Outputs (bf16 conv, fp32 scores) leave in batched DMAs on both HWDGE
queues.  Softmax + gather + final multiply happen on the host (cheap,
like the baseline's gather).  Sharding: data-parallel over batch,
2 batches per core on 8 cores; params replicated.
"""

import sys

import numpy as np

if "/opt/trn_rl_repo" not in sys.path:
    sys.path.insert(0, "/opt/trn_rl_repo")

B, T, D, W, F, U = 16, 260, 32, 5, 256, 128
TC = T - W + 1  # 256
NCORES = 8
BPC = B // NCORES  # batches per core = 2

# tanh(x) ~= C_LIN*x + sum_m A_FIT[m] * sin((m+1)*OM1*x), |x| <= 7
OM1 = 0.65
A_FIT = [0.53877437, 0.1715707, 0.06058703, 0.02243426, 0.008573]
KAPPA = [1.0, 2.0, 1.0, 4.0, 1.0]  # product-form scale absorbed into folds
C_LIN = 0.20662367
M = len(A_FIT)

_PROGRAM = None


def _build_program():
    import concourse.bacc as bacc
    import concourse.tile as tile
    from concourse import mybir

    f32 = mybir.dt.float32
    bf16 = mybir.dt.bfloat16
    i32 = mybir.dt.int32
    AF = mybir.ActivationFunctionType
    ALU = mybir.AluOpType
    PI_2 = 1.5707963267948966

    nc = bacc.Bacc()

    x128_in = nc.declare_dram_parameter("x128", [BPC, 128, TC], bf16, isOutput=False)
    xT_in = nc.declare_dram_parameter("xT_loc", [BPC, D, T], bf16, isOutput=False)
    ckA_in = nc.declare_dram_parameter("ckA", [128, 2, 128], bf16, isOutput=False)
    ckB_in = nc.declare_dram_parameter("ckB", [D, 2, 128], bf16, isOutput=False)
    w12_in = nc.declare_dram_parameter("w12", [128, 2, 2, U], bf16, isOutput=False)
    # smalls columns: 0 cb_c0, 1 cb_c1, 2 b1, 3 b2, 4 cV, 5.. kappa_m*a_m*V
    SM = 5 + M
    sm_in = nc.declare_dram_parameter("smalls", [128, SM], f32, isOutput=False)

    convT_out = nc.declare_dram_parameter(
        "convT_out", [BPC, 2, 128, TC], bf16, isOutput=True
    )
    scoreT_out = nc.declare_dram_parameter(
        "scoreT_out", [BPC, 2, 128, TC], f32, isOutput=True
    )

    with tile.TileContext(nc) as tc:
        with (
            tc.tile_pool(name="const", bufs=1) as const,
            tc.tile_pool(name="cvp", bufs=1) as cvp,
            tc.tile_pool(name="qvp", bufs=1) as qvp,
            tc.tile_pool(name="wrp", bufs=2) as wrp,
            tc.tile_pool(name="ft", bufs=1) as ft,
            tc.tile_pool(name="sc", bufs=1) as sc,
            tc.tile_pool(name="ps1", bufs=2, space="PSUM") as ps1,
            tc.tile_pool(name="ps2", bufs=2, space="PSUM") as ps2,
            tc.tile_pool(name="pss", bufs=4, space="PSUM") as pss,
        ):
            HTC = BPC * TC  # 512: q-half / v-half boundary

            # ---- earliest vector op: the dummy-matmul source ----
            wsrc = const.tile([128, TC], bf16, tag="wsrc")
            nc.vector.memset(wsrc[:], 0.0)

            # ---- input DMAs across both HWDGE queues; x128 split per batch ----
            x128_sb = const.tile([128, BPC, TC], bf16, tag="x128")
            nc.sync.dma_start(out=x128_sb[:, 0, :], in_=x128_in[0])
            ckA_sb = const.tile([128, 2, 128], bf16, tag="ckA")
            nc.scalar.dma_start(out=ckA_sb[:], in_=ckA_in[:])
            xT_all = const.tile([D, BPC, T], bf16, tag="xT")
            nc.sync.dma_start(
                out=xT_all[:], in_=xT_in[:, :, :].rearrange("i d t -> d i t")
            )
            ckB_sb = const.tile([D, 2, 128], bf16, tag="ckB")
            nc.scalar.dma_start(out=ckB_sb[:], in_=ckB_in[:])
            nc.sync.dma_start(out=x128_sb[:, 1, :], in_=x128_in[1])
            sm_sb = const.tile([128, SM], f32, tag="sm")
            nc.scalar.dma_start(out=sm_sb[:], in_=sm_in[:])
            w12_sb = const.tile([128, 2, 2, U], bf16, tag="w12")
            nc.sync.dma_start(out=w12_sb[:], in_=w12_in[:])

            # trig table load rides ahead of this first Sin
            warm2 = const.tile([1, 1], f32, tag="warm2")
            nc.scalar.activation(out=warm2[:], in_=wsrc[0:1, 0:1], func=AF.Sin)

            # ---- PE HAM warm-up spin (no DMA dependencies) ----
            for _ in range(5):
                ps_w = ps1.tile([128, TC], f32, tag="mm1")
                nc.tensor.matmul(
                    out=ps_w[:], lhsT=wsrc[:, :128], rhs=wsrc[:],
                    start=True, stop=True,
                )

            ones_sb = const.tile([128, 128], bf16, tag="ones")
            nc.vector.memset(ones_sb[:], 1.0)
            pi2_sb = const.tile([128, 1], f32, tag="pi2")
            nc.vector.memset(pi2_sb[:], PI_2)
            zero_sb = const.tile([128, 1], f32, tag="zero")
            nc.vector.memset(zero_sb[:], 0.0)

            # ---- phase 1: conv (2 MMs/chunk), q, v ----
            # QV free layout: [q0 | q1 | v0 | v1] blocks of TC
            QV = qvp.tile([128, 2 * HTC], f32, tag="QV")
            rhsL = qvp.tile([128, BPC, TC], bf16, tag="rhsL")
            conv_bf = []
            for i in range(BPC):
                cvb = cvp.tile([128, 2, TC], bf16, tag=f"convbf{i}", name=f"cvb{i}")
                for c in range(2):
                    ps_cv = ps1.tile([128, TC], f32, tag="mm1")
                    nc.tensor.matmul(
                        out=ps_cv[:], lhsT=ckA_sb[:, c, :], rhs=x128_sb[:, i, :],
                        start=True, stop=False,
                    )
                    nc.tensor.matmul(
                        out=ps_cv[:], lhsT=ckB_sb[:, c, :],
                        rhs=xT_all[:, i, 4 : 4 + TC],
                        start=False, stop=True,
                    )
                    nc.vector.tensor_scalar(
                        out=cvb[:, c, :], in0=ps_cv[:],
                        scalar1=sm_sb[:, c : c + 1], scalar2=0.0,
                        op0=ALU.add, op1=ALU.max,
                    )
                nc.sync.dma_start(
                    out=convT_out[i].rearrange("c p t -> p c t"), in_=cvb[:]
                )
                conv_bf.append(cvb)

            for i in range(BPC):
                ps_qv = ps2.tile([U, 2, TC], f32, tag="mmqv")
                for s in range(2):
                    for c in range(2):
                        nc.tensor.matmul(
                            out=ps_qv[:, s, :],
                            lhsT=w12_sb[:, s, c, :],
                            rhs=conv_bf[i][:, c, :],
                            start=(c == 0),
                            stop=(c == 1),
                        )
                for s in range(2):
                    blk = s * BPC + i
                    nc.scalar.activation(
                        out=QV[:, blk * TC : (blk + 1) * TC], in_=ps_qv[:, s, :],
                        func=AF.Identity, bias=sm_sb[:, 2 + s : 3 + s],
                    )
                nc.vector.tensor_scalar_mul(
                    out=rhsL[:, i, :],
                    in0=QV[:, (BPC + i) * TC : (BPC + i + 1) * TC],
                    scalar1=sm_sb[:, 4:5],
                )

            # keep the PE spinning (HAM warm); rhs=QV anchors after phase 1
            for _ in range(8):
                ps_w = ps1.tile([128, TC], f32, tag="mm1")
                nc.tensor.matmul(
                    out=ps_w[:], lhsT=QV[:, 0:128], rhs=QV[:, 0:TC],
                    start=True, stop=True,
                )

            # ---- phase 2: m=1,5 via ACT (+wrap for cos1/sin5/cos5);
            #      m=2,3,4 via DVE multiple-angle products ----
            def wrap(om):
                P = 2.0 * np.pi / om
                Phi = float(np.float32(P))
                Plo = float(np.float64(P) - np.float64(Phi))
                kt = wrp.tile([128, 2 * HTC], i32, tag="k", name=f"k{om:.2f}")
                nc.vector.tensor_scalar(
                    out=kt[:], in0=QV[:], scalar1=float(1.0 / P),
                    scalar2=0.125, op0=ALU.mult, op1=ALU.add,
                )
                wt = wrp.tile([128, 2 * HTC], f32, tag="w", name=f"w{om:.2f}")
                nc.vector.cody_waite_cascade(
                    out=wt[:], x=QV[:], k=kt[:], c1=Phi, c2=Plo, c3=0.0
                )
                return wt

            bft = lambda name: ft.tile([128, 2 * HTC], bf16, tag=name, name=name)

            om5 = 5.0 * OM1
            s1 = bft("s1")
            nc.scalar.activation(out=s1[:], in_=QV[:], func=AF.Sin, scale=OM1,
                                 bias=zero_sb[:])
            w1t = wrap(OM1)
            u = bft("u")
            nc.scalar.activation(out=u[:], in_=s1[:], func=AF.Square,
                                 bias=zero_sb[:])
            c1 = bft("c1")
            nc.scalar.activation(out=c1[:], in_=w1t[:], func=AF.Sin, scale=OM1,
                                 bias=pi2_sb[:])
            w5t = wrap(om5)
            s5 = bft("s5")
            nc.scalar.activation(out=s5[:], in_=w5t[:], func=AF.Sin, scale=om5,
                                 bias=zero_sb[:])
            c5 = bft("c5")
            nc.scalar.activation(out=c5[:], in_=w5t[:], func=AF.Sin, scale=om5,
                                 bias=pi2_sb[:])

            p = bft("p")
            nc.vector.tensor_mul(out=p[:], in0=s1[:], in1=c1[:])
            c2t = bft("c2t")
            nc.vector.tensor_scalar(out=c2t[:], in0=u[:], scalar1=-2.0,
                                    scalar2=1.0, op0=ALU.mult, op1=ALU.add)
            t3a = bft("t3a")
            nc.vector.tensor_scalar(out=t3a[:], in0=u[:], scalar1=-4.0,
                                    scalar2=3.0, op0=ALU.mult, op1=ALU.add)
            t3b = bft("t3b")
            nc.vector.tensor_scalar(out=t3b[:], in0=u[:], scalar1=-4.0,
                                    scalar2=1.0, op0=ALU.mult, op1=ALU.add)
            s3 = bft("s3")
            nc.vector.tensor_mul(out=s3[:], in0=s1[:], in1=t3a[:])
            c3 = bft("c3")
            nc.vector.tensor_mul(out=c3[:], in0=c1[:], in1=t3b[:])
            s4 = bft("s4")
            nc.vector.tensor_mul(out=s4[:], in0=p[:], in1=c2t[:])
            p2 = bft("p2")
            nc.scalar.activation(out=p2[:], in_=p[:], func=AF.Square,
                                 bias=zero_sb[:])
            c4t = bft("c4t")
            nc.vector.tensor_scalar(out=c4t[:], in0=p2[:], scalar1=-8.0,
                                    scalar2=1.0, op0=ALU.mult, op1=ALU.add)

            SINS = [s1, p, s3, s4, s5]
            COSS = [c1, c2t, c3, c4t, c5]

            # ---- phase 3: folds + score matmuls, readiness order ----
            groups = [(i, ch) for i in range(BPC) for ch in range(2)]
            psS = {
                g: pss.tile([128, TC], f32, tag="score", name=f"psS{g[0]}{g[1]}")
                for g in groups
            }
            morder = [0, 1, 2, 4, 3]
            for k, m in enumerate(morder):
                lhs_s = ft.tile([128, HTC], bf16, tag=f"lhs{m}", name=f"lhs{m}")
                nc.vector.tensor_scalar_mul(
                    out=lhs_s[:], in0=SINS[m][:, 0:HTC],
                    scalar1=sm_sb[:, 5 + m : 6 + m],
                )
                lhs_c = ft.tile([128, HTC], bf16, tag=f"lhc{m}", name=f"lhc{m}")
                nc.vector.tensor_scalar_mul(
                    out=lhs_c[:], in0=COSS[m][:, 0:HTC],
                    scalar1=sm_sb[:, 5 + m : 6 + m],
                )
                for i in range(BPC):
                    voff = HTC + i * TC
                    for ch in range(2):
                        nc.tensor.matmul(
                            out=psS[(i, ch)][:],
                            lhsT=lhs_s[:, i * TC + ch * 128 : i * TC + (ch + 1) * 128],
                            rhs=COSS[m][:, voff : voff + TC],
                            start=(k == 0),
                            stop=False,
                        )
                        nc.tensor.matmul(
                            out=psS[(i, ch)][:],
                            lhsT=lhs_c[:, i * TC + ch * 128 : i * TC + (ch + 1) * 128],
                            rhs=SINS[m][:, voff : voff + TC],
                            start=False,
                            stop=False,
                        )

            ssb = sc.tile([128, BPC, 2, TC], f32, tag="ssb")
            for n, (i, ch) in enumerate(groups):
                nc.tensor.matmul(
                    out=psS[(i, ch)][:],
                    lhsT=ones_sb[:],
                    rhs=rhsL[:, i, :],
                    start=False,
                    stop=True,
                )
                eng = nc.scalar if n < 2 else nc.vector
                if n < 2:
                    nc.scalar.copy(out=ssb[:, i, ch, :], in_=psS[(i, ch)][:])
                else:
                    nc.vector.tensor_copy(out=ssb[:, i, ch, :], in_=psS[(i, ch)][:])
            nc.sync.dma_start(
                out=scoreT_out[0].rearrange("c p t -> p c t"), in_=ssb[:, 0]
            )
            nc.scalar.dma_start(
                out=scoreT_out[1].rearrange("c p t -> p c t"), in_=ssb[:, 1]
            )

    nc.compile()
    return nc


def _get_program():
    global _PROGRAM
    if _PROGRAM is None:
        _PROGRAM = _build_program()
    return _PROGRAM


def _install_trace_shims():
    """This image's antenv lacks axon_hooks; register the ctypes NTFF hook
    manually and stub out the S3 artifact upload."""
    import types

    try:
        from antenv import axon_hooks  # noqa: F401
        return
    except ImportError:
        pass
    from trn_agent_boot.trn_boot import _ntff_profile_via_ctypes

    hook = _ntff_profile_via_ctypes("/opt/axon/libaxon_pjrt.so")
    mod = types.ModuleType("antenv.axon_hooks")
    mod.get_axon_ntff_profile_hook = lambda: hook
    mod.set_axon_ntff_profile_hook = lambda h: None
    sys.modules["antenv.axon_hooks"] = mod

    import concourse.bass_utils as bu

    bu.upload_artifacts = lambda tmpdir: f"local:{tmpdir}"


def run(inputs, trace=False, trace_kwargs=None):
    """Run the SPMD kernel. Returns (output, BassKernelResults)."""
    import ml_dtypes

    from concourse.bass_utils import run_bass_kernel_spmd

    if trace:
        _install_trace_shims()

    nc = _get_program()
    bfdt = ml_dtypes.bfloat16

    x = np.asarray(inputs["x"], dtype=np.float32)
    ck = np.asarray(inputs["conv_kernel"], dtype=np.float32).reshape(W, D, F)
    cb = np.asarray(inputs["conv_bias"], dtype=np.float32)
    w1 = np.asarray(inputs["W1"], dtype=np.float32)
    b1 = np.asarray(inputs["b1"], dtype=np.float32)
    w2 = np.asarray(inputs["W2"], dtype=np.float32)
    b2 = np.asarray(inputs["b2"], dtype=np.float32)
    v = np.asarray(inputs["V"], dtype=np.float32).reshape(U)

    xT = np.ascontiguousarray(x.transpose(0, 2, 1).astype(bfdt))  # (B, D, T)
    arr = np.stack([x[:, w : w + TC, :] for w in range(4)], axis=2)  # (B,TC,4,D)
    x128 = np.ascontiguousarray(
        arr.reshape(B, TC, 128).transpose(0, 2, 1).astype(bfdt)
    )  # (B, 128, TC)
    ckA = np.ascontiguousarray(ck[:4].reshape(128, 2, 128).astype(bfdt))
    ckB = np.ascontiguousarray(ck[4].reshape(D, 2, 128).astype(bfdt))
    w12 = np.ascontiguousarray(
        np.stack(
            [w1.reshape(2, 128, U).transpose(1, 0, 2),
             w2.reshape(2, 128, U).transpose(1, 0, 2)],
            axis=1,
        ).astype(bfdt)
    )  # (128, 2src, 2chunk, U)
    smalls = np.zeros((128, 5 + M), dtype=np.float32)
    smalls[:, 0:2] = cb.reshape(2, 128).T
    smalls[:, 2] = b1
    smalls[:, 3] = b2
    smalls[:, 4] = C_LIN * v
    ka = np.asarray(A_FIT, dtype=np.float32) * np.asarray(KAPPA, dtype=np.float32)
    smalls[:, 5:] = v[:, None] * ka[None, :]
    smalls = np.ascontiguousarray(smalls)

    in_maps = []
    for c in range(NCORES):
        in_maps.append(
            {
                "x128": np.ascontiguousarray(x128[c * BPC : (c + 1) * BPC]),
                "xT_loc": np.ascontiguousarray(xT[c * BPC : (c + 1) * BPC]),
                "ckA": ckA,
                "ckB": ckB,
                "w12": w12,
                "smalls": smalls,
            }
        )

    kw = {}
    if trace:
        kw["trace"] = True
        if trace_kwargs:
            kw["trace_kwargs"] = trace_kwargs
    res = run_bass_kernel_spmd(nc, in_maps, list(range(NCORES)), **kw)

    # ---- host-side gather / softmax / final multiply ----
    convT = np.stack(
        [np.asarray(r["convT_out"], dtype=np.float32) for r in res.results]
    )
    scoreT = np.stack([r["scoreT_out"] for r in res.results])  # (8, 2, 2, 128, 256)
    conv = convT.reshape(B, F, TC).transpose(0, 2, 1)  # (B, t, f)
    score = scoreT.reshape(B, TC, TC)  # (B, t, j)

    score = score - score.max(axis=2, keepdims=True)
    np.exp(score, out=score)
    score /= score.sum(axis=2, keepdims=True)  # attn (B, t, j)

    # out[b', t', f] = conv[b', t', f] * attn[t' % 16, b'*16 + t'//16, f]
    tp = np.arange(TC)
    bp = np.arange(B)[:, None]
    att_s = score[(tp % B)[None, :], bp * (TC // B) + (tp // B)[None, :], :]
    out = (conv * att_s).astype(np.float32)
    return out, res


def kernel(**inputs) -> np.ndarray:
    out, _ = run(inputs, trace=False)
    return out
